# revision 42
# baseline (speedup 1.0000x reference)
"""GRU cell kernel for Trainium2, data-parallel across 8 NeuronCores.

Reference computation (per batch row):
    concat = [h_prev, x]                       # [B, 2048]
    z = sigmoid(concat @ W_z.T + b_z)          # [B, 1024]
    r = sigmoid(concat @ W_r.T + b_r)
    h_tilde = tanh([r*h_prev, x] @ W_h.T + b_h)
    h = (1-z)*h_prev + z*h_tilde

Sharding: batch dim (8192) split 1024/core; weights replicated.
Layout on device is feature-major ([feature, batch]) so the batch is the
matmul moving dimension (N=512 per PSUM bank) and the hidden units are the
PSUM partition dim. Host transposes in/out.

Precision modes (mm_dtype):
  fp8all — all three gates fp8 e4m3 DoubleRow (2x PE rate: 213ns per
           512-wide 256-K matmul vs 54.6us/gate in bf16). rel err 1.76e-2.
           DEFAULT: ~105us vs 194us for the tuned bf16 baseline.
  mixed  — z/r gates fp8, candidate (tanh) gate bf16. rel err 1.07e-2 but
           ~175us: running fp8 AND bf16 phases on all 8 cores trips a
           package-level clock throttle (~2.07GHz vs 2.37), so the safer
           numerics cost more than their cycle count suggests. Pure-fp8
           does not throttle.
  Scales: weights x256, activations x16; the 1/4096 descale is folded into
  the ACT scale operand. fp8 moving/stationary pairs contract 256 K per
  pass ([128, 2, M] x [128, 2, N]).
"""

import numpy as np

import concourse.bacc as bacc
import concourse.bass as bass
import concourse.mybir as mybir
import concourse.tile as tile
from concourse import bass_utils

P = 128
B = 8192
I = 1024
H = 1024
K = I + H            # 2048 contraction
NCORES = 8
BS = B // NCORES     # 1024 batch rows per core
MT = H // P          # 8 m-tiles (hidden units)
KT = K // P          # 16 k-chunks (bf16)
KT2 = KT // 2        # 8 doubled k-chunks (fp8 DoubleRow)
KH2 = H // P // 2    # 4 doubled chunks in the h/rh half
NFREE = 512          # matmul moving free dim (one PSUM bank of fp32)
NT = BS // NFREE     # 2 n-tiles per core

ASC = 16.0           # activation quantization scale (fp8)
WS = 256.0           # weight quantization scale (fp8)
INV = 1.0 / (ASC * WS)

F32 = mybir.dt.float32
BF16 = mybir.dt.bfloat16
FP8 = mybir.dt.float8e4

AF = mybir.ActivationFunctionType
DR = mybir.MatmulPerfMode.DoubleRow


def build_kernel(mode: str = "fp8all"):
    """Build the per-core Bass kernel. Returns compiled nc."""
    assert mode in ("mixed", "fp8all")
    fp8_h = mode == "fp8all"
    nc = bacc.Bacc("TRN2", target_bir_lowering=False, debug=False)

    # DRAM I/O (per-core shapes).
    x8 = nc.dram_tensor("x8", [I, BS], FP8, kind="ExternalInput").ap()
    h8 = nc.dram_tensor("h8", [H, BS], FP8, kind="ExternalInput").ap()
    hb = nc.dram_tensor("hb", [H, BS], BF16, kind="ExternalInput").ap()
    if not fp8_h:
        xb = nc.dram_tensor("xb", [I, BS], BF16, kind="ExternalInput").ap()
    Wz = nc.dram_tensor("Wz", [MT, P, KT2, 2, P], FP8, kind="ExternalInput").ap()
    Wr = nc.dram_tensor("Wr", [MT, P, KT2, 2, P], FP8, kind="ExternalInput").ap()
    if fp8_h:
        Wh = nc.dram_tensor("Wh", [MT, P, KT2, 2, P], FP8,
                            kind="ExternalInput").ap()
    else:
        Wh = nc.dram_tensor("Wh", [MT, P, K], BF16, kind="ExternalInput").ap()
    bz = nc.dram_tensor("bz", [P, MT], F32, kind="ExternalInput").ap()
    br = nc.dram_tensor("br", [P, MT], F32, kind="ExternalInput").ap()
    bh = nc.dram_tensor("bh", [P, MT], F32, kind="ExternalInput").ap()
    out = nc.dram_tensor("out", [H, BS], BF16, kind="ExternalOutput").ap()

    with tile.TileContext(nc) as tc:
        with (
            tc.tile_pool(name="acts", bufs=1) as acts,
            tc.tile_pool(name="gates", bufs=1) as gates,
            tc.tile_pool(name="wpool", bufs=8) as wpool,
            tc.tile_pool(name="opool", bufs=6) as opool,
            tc.tile_pool(name="ppool", bufs=8, space="PSUM") as ppool,
        ):
            bz_sb = acts.tile([P, MT], F32)
            br_sb = acts.tile([P, MT], F32)
            bh_sb = acts.tile([P, MT], F32)

            # Weight tiles rotate through one 8-buffer pool (fp8all): the
            # 9th+ weight DMA must wait for the matmuls consuming the
            # buffer's previous occupant. This backpressure stops the Wz/Wh
            # prefetch (needed only 40/70us in) from saturating the DMA
            # queues while the r-phase act loads are on the critical path.
            def wtile(nm):
                if fp8_h:
                    return wpool.tile([P, KT2, 2, P], FP8, tag="w", name=nm)
                return wpool.tile([P, KT2, 2, P], FP8, name=nm)

            wr_sb = {mt: wtile(f"wr{mt}") for mt in range(MT)}

            # Persistent activations, feature-major: [p, ko, batch]
            x8_sb = acts.tile([P, I // P, BS], FP8)
            h8_sb = acts.tile([P, H // P, BS], FP8)
            hb_sb = acts.tile([P, H // P, BS], BF16)
            xb_sb = (acts.tile([P, I // P, BS], BF16, name="xb_sb")
                     if not fp8_h else None)
            x8_r = x8.rearrange("(ko p) b -> p ko b", p=P)
            h8_r = h8.rearrange("(ko p) b -> p ko b", p=P)
            hb_r = hb.rearrange("(ko p) b -> p ko b", p=P)
            xb_r = xb.rearrange("(ko p) b -> p ko b", p=P) if not fp8_h else None
            n0 = slice(0, NFREE)
            n1 = slice(NFREE, BS)

            # PE p-state warm-up: the tensor engine runs at reduced clock
            # (~250ns/512-row matmul vs 213) unless warmed; without this
            # block the ENTIRE kernel executes at the lower clock. Chew
            # through the warm-up on garbage matmuls (no DMA deps, results
            # never read) while the real weights/acts are still in flight.
            wu_w = acts.tile([P, 2, P], FP8, name="wu_w")
            wu_m = acts.tile([P, 2, 256], FP8, name="wu_m")
            nc.vector.memset(wu_w[:], 0)
            nc.vector.memset(wu_m[:], 0)
            wu_ps = ppool.tile([P, NFREE], F32, tag="ps", name="wu_ps")
            for i in range(44):
                nc.tensor.matmul(wu_ps[:, :256], wu_w[:], wu_m[:],
                                 start=True, stop=True, perf_mode=DR)

            # Early DMA bandwidth is descriptor-supply-limited PER RING
            # (~175KB/us each, strictly FIFO within a ring), so the
            # ramp-critical bytes are spread over all three rings in
            # consumption order. The scalar ring spins up ~1-3us after
            # sync; gpsimd ~2us after.
            nc.sync.dma_start(wr_sb[0][:], Wr[0])
            nc.sync.dma_start(h8_sb[:, :4, n0], h8_r[:, :4, n0])
            nc.sync.dma_start(h8_sb[:, 4:, n0], h8_r[:, 4:, n0])
            nc.sync.dma_start(x8_sb[:, :, n0], x8_r[:, :, n0])
            nc.sync.dma_start(h8_sb[:, :, n1], h8_r[:, :, n1])
            nc.sync.dma_start(x8_sb[:, :, n1], x8_r[:, :, n1])
            nc.sync.dma_start(hb_sb[:, :4, :], hb_r[:, :4, :])
            nc.sync.dma_start(hb_sb[:, 4:, :], hb_r[:, 4:, :])

            nc.scalar.dma_start(wr_sb[1][:], Wr[1])
            nc.scalar.dma_start(wr_sb[2][:], Wr[2])
            nc.scalar.dma_start(wr_sb[3][:], Wr[3])
            # Pre-warm the ACT sigmoid/tanh table set during the DMA fill so
            # the first real sigmoid doesn't pay the ~2.7us ACT_TABLE_LOAD.
            warm = acts.tile([P, 1], F32)
            nc.scalar.activation(warm[:], warm[:], AF.Sigmoid)
            nc.scalar.dma_start(br_sb[:], br)
            nc.scalar.dma_start(bz_sb[:], bz)
            nc.scalar.dma_start(bh_sb[:], bh)

            # Remaining weights ride the GpSimd SWDGE queue (spins up ~6us,
            # idle otherwise); wz/wh are held back by the wpool rotation.
            for mt in range(4, MT):
                nc.gpsimd.dma_start(wr_sb[mt][:], Wr[mt])
            wz_sb = {}
            for mt in range(MT):
                wz_sb[mt] = wpool.tile([P, KT2, 2, P], FP8,
                                       tag="w" if fp8_h else None,
                                       name=f"wz{mt}")
                nc.gpsimd.dma_start(wz_sb[mt][:], Wz[mt])
            wh_sb = {}
            for mt in range(MT):
                if fp8_h:
                    wh_sb[mt] = wpool.tile([P, KT2, 2, P], FP8, tag="w",
                                           name=f"wh{mt}")
                else:
                    wh_sb[mt] = wpool.tile([P, K], BF16, name=f"wh{mt}")
                nc.gpsimd.dma_start(wh_sb[mt][:], Wh[mt])
            if not fp8_h:
                nc.gpsimd.dma_start(xb_sb[:, :4, :], xb_r[:, :4, :])
                nc.gpsimd.dma_start(xb_sb[:, 4:, :], xb_r[:, 4:, :])

            # Gate results, feature-major. v = (1-z)*h_prev is precomputed
            # during the (PE-bound, DVE-idle) z phase so the final combine
            # after the last matmul is only mul+add.
            z_sb = gates.tile([P, MT, BS], BF16)
            v_sb = gates.tile([P, MT, BS], BF16)
            rh_sb = gates.tile([P, MT, BS], FP8 if fp8_h else BF16)

            def mov8(k2, base, width):
                """fp8 moving operand [128, 2, width] for doubled chunk k2
                of the [h_prev, x] concat."""
                if k2 < KH2:
                    return h8_sb[:, 2 * k2:2 * k2 + 2, base:base + width]
                kx = k2 - KH2
                return x8_sb[:, 2 * kx:2 * kx + 2, base:base + width]

            def mov8_h(k2, base, width):
                """fp8 moving operand for the candidate gate ([rh, x])."""
                if k2 < KH2:
                    return rh_sb[:, 2 * k2:2 * k2 + 2, base:base + width]
                kx = k2 - KH2
                return x8_sb[:, 2 * kx:2 * kx + 2, base:base + width]

            def movb_h(k, base, width):
                """bf16 moving operand [128, width] for the candidate gate."""
                if k < H // P:
                    return rh_sb[:, k, base:base + width]
                return xb_sb[:, k - H // P, base:base + width]

            def fp8_group(ps, w_sb, mov, base, width):
                for k2 in range(KT2):
                    nc.tensor.matmul(
                        ps, w_sb[:, k2], mov(k2, base, width),
                        start=(k2 == 0), stop=(k2 == KT2 - 1), perf_mode=DR)

            # ---- R gate (fp8) ----
            # 8-wide mega-ramp: open ALL 8 PSUM banks on (mt0-7, n=0) and
            # run the entire h_prev half (32 matmuls, ~6.8us of runway)
            # before the first x chunk is touched. This moves the x8-n0
            # deadline from ramp+3.4us to ramp+6.8us — past its DMA arrival
            # even on a slow ring day — and staggers the weight needs to
            # match their ring order: wr0-3 (sync/scalar) first, wr4-7
            # (gpsimd) from ~+3.4us, g6/g7 last.
            NG = 8
            pss = [ppool.tile([P, NFREE], F32, tag="ps", name=f"psri{g}")
                   for g in range(NG)]
            ramp = ([(k2, g) for k2 in range(KH2) for g in range(4)]
                    + [(k2, g) for k2 in range(KH2) for g in (4, 5)]
                    + [(k2, g) for k2 in range(KH2) for g in (6, 7)]
                    + [(k2, g) for k2 in range(KH2, KT2) for g in range(NG)])
            for k2, g in ramp:
                nc.tensor.matmul(
                    pss[g], wr_sb[g][:, k2], mov8(k2, 0, NFREE),
                    start=(k2 == 0), stop=(k2 == KT2 - 1), perf_mode=DR)

            def finish_r(mt, ps, base, width):
                ns = slice(base, base + width)
                r_tmp = opool.tile([P, width], BF16, tag="rt")
                nc.scalar.activation(r_tmp, ps, AF.Sigmoid,
                                     bias=br_sb[:, mt:mt + 1], scale=INV)
                if fp8_h:
                    # rh = fp8(16 * r * h_prev): one fused DVE op
                    nc.vector.scalar_tensor_tensor(
                        rh_sb[:, mt, ns], r_tmp, ASC, hb_sb[:, mt, ns],
                        op0=mybir.AluOpType.mult, op1=mybir.AluOpType.mult)
                else:
                    nc.vector.tensor_mul(
                        rh_sb[:, mt, ns], r_tmp, hb_sb[:, mt, ns])

            for g in range(NG):
                finish_r(g, pss[g], 0, NFREE)
            r_plan = [(mt, 1) for mt in range(MT)]
            for mt, n in r_plan:
                base = n * NFREE
                ps = ppool.tile([P, NFREE], F32, tag="ps", name=f"psr{mt}_{n}")
                fp8_group(ps, wr_sb[mt], mov8, base, NFREE)
                finish_r(mt, ps, base, NFREE)

            # ---- Z gate (fp8) ----
            for mt in range(MT):
                for n in range(NT):
                    base = n * NFREE
                    ns = slice(base, base + NFREE)
                    ps = ppool.tile([P, NFREE], F32, tag="ps",
                                    name=f"psz{mt}_{n}")
                    fp8_group(ps, wz_sb[mt], mov8, base, NFREE)
                    nc.scalar.activation(z_sb[:, mt, ns], ps, AF.Sigmoid,
                                         bias=bz_sb[:, mt:mt + 1], scale=INV)
                    u = opool.tile([P, NFREE], BF16, tag="u")
                    nc.vector.tensor_mul(u, z_sb[:, mt, ns], hb_sb[:, mt, ns])
                    nc.vector.tensor_sub(v_sb[:, mt, ns], hb_sb[:, mt, ns], u)

            # ---- H (candidate) gate ----
            for mt in range(MT):
                for n in range(NT):
                    # Split the very last group so its activation+combine
                    # +store chain pipelines instead of sitting fully
                    # exposed after the final matmul.
                    last = mt == MT - 1 and n == NT - 1
                    nsub = 2 if last else 1
                    width = NFREE // nsub
                    for s in range(nsub):
                        base = n * NFREE + s * width
                        ns = slice(base, base + width)
                        ps = ppool.tile([P, width], F32, tag="ps",
                                        name=f"psh{mt}_{n}_{s}")
                        if fp8_h:
                            fp8_group(ps, wh_sb[mt], mov8_h, base, width)
                        else:
                            for k in range(KT):
                                nc.tensor.matmul(
                                    ps, wh_sb[mt][:, k * P:(k + 1) * P],
                                    movb_h(k, base, width),
                                    start=(k == 0), stop=(k == KT - 1))
                        ht = opool.tile([P, width], BF16, tag="ht",
                                        name=f"ht{mt}_{n}_{s}")
                        nc.scalar.activation(
                            ht, ps, AF.Tanh, bias=bh_sb[:, mt:mt + 1],
                            scale=INV if fp8_h else 1.0)
                        # h = z*h_tilde + v, all bf16
                        nc.vector.tensor_mul(ht, ht, z_sb[:, mt, ns])
                        nc.vector.tensor_add(ht, ht, v_sb[:, mt, ns])
                        # Stores alternate between the sync and scalar HWDGE
                        # rings (both idle by the h phase) so the final
                        # tiles' stores drain in parallel. gpsimd is avoided:
                        # its FIFO still holds backpressured Wh loads.
                        eng = (nc.sync, nc.scalar)[(mt * NT + n + s) % 2]
                        eng.dma_start(out[mt * P:(mt + 1) * P, ns], ht)

    nc.compile()
    return nc


def _prep_inputs(x, h_prev, W_z, b_z, W_r, b_r, W_h, b_h, mode="mixed"):
    """Host-side relayout + quantization."""
    import ml_dtypes
    E4 = ml_dtypes.float8_e4m3
    BF = ml_dtypes.bfloat16
    fp8_h = mode == "fp8all"

    def prep_w8(W):
        # want w[mt, p, k2, i, m] = 256*W[mt*128+m, (k2*2+i)*128+p]
        W5 = W.reshape(MT, P, KT2, 2, P)          # [mt, m, k2, i, p]
        W5 = np.ascontiguousarray(W5.transpose(0, 4, 2, 3, 1))
        return np.clip(W5 * WS, -240, 240).astype(E4)

    def prep_wb(W):
        # want w[mt, p, ko*128+m] = W[mt*128+m, ko*128+p]
        W4 = W.reshape(MT, P, KT, P)              # [mt, m, ko, p]
        return np.ascontiguousarray(
            W4.transpose(0, 3, 2, 1)).reshape(MT, P, K).astype(BF)

    def prep_b(b):
        return np.ascontiguousarray(b.reshape(MT, P).T)

    xT = np.ascontiguousarray(x.T)                # [I, B] f32
    hT = np.ascontiguousarray(h_prev.T)           # [H, B] f32
    x8 = np.clip(xT * ASC, -240, 240).astype(E4)
    h8 = np.clip(hT * ASC, -240, 240).astype(E4)
    xb = xT.astype(BF)
    hb = hT.astype(BF)
    shared = {
        "Wz": prep_w8(W_z), "Wr": prep_w8(W_r),
        "Wh": prep_w8(W_h) if fp8_h else prep_wb(W_h),
        "bz": prep_b(b_z), "br": prep_b(b_r), "bh": prep_b(b_h),
    }
    in_maps = []
    for c in range(NCORES):
        bs = slice(c * BS, (c + 1) * BS)
        m = dict(shared)
        m["x8"] = np.ascontiguousarray(x8[:, bs])
        m["h8"] = np.ascontiguousarray(h8[:, bs])
        m["hb"] = np.ascontiguousarray(hb[:, bs])
        if not fp8_h:
            m["xb"] = np.ascontiguousarray(xb[:, bs])
        in_maps.append(m)
    return in_maps


def run(inputs, mm_dtype="fp8all", trace=False, **run_kwargs):
    """Compile + run on 8 cores. Returns (output [B,H] f32, BassKernelResults)."""
    nc = build_kernel(mm_dtype)
    in_maps = _prep_inputs(**inputs, mode=mm_dtype)
    res = bass_utils.run_bass_kernel_spmd(
        nc, in_maps, core_ids=list(range(NCORES)), trace=trace, **run_kwargs)
    outT = np.concatenate(
        [res.results[c]["out"].astype(np.float32) for c in range(NCORES)],
        axis=1)  # [H, B]
    return np.ascontiguousarray(outT.T), res


def kernel(**inputs) -> np.ndarray:
    import time as _time
    try:
        out, _ = run(inputs)
    except Exception:
        # The axon-tunneled device occasionally reports a transient
        # "unrecoverable" state right after a crashed session; a fresh
        # attempt after a short pause recovers.
        _time.sleep(15)
        out, _ = run(inputs)
    return out


# revision 43
# speedup vs baseline: 1.0014x; 1.0014x over previous
"""GRU cell kernel for Trainium2, data-parallel across 8 NeuronCores.

Reference computation (per batch row):
    concat = [h_prev, x]                       # [B, 2048]
    z = sigmoid(concat @ W_z.T + b_z)          # [B, 1024]
    r = sigmoid(concat @ W_r.T + b_r)
    h_tilde = tanh([r*h_prev, x] @ W_h.T + b_h)
    h = (1-z)*h_prev + z*h_tilde

Sharding: batch dim (8192) split 1024/core; weights replicated.
Layout on device is feature-major ([feature, batch]) so the batch is the
matmul moving dimension (N=512 per PSUM bank) and the hidden units are the
PSUM partition dim. Host transposes in/out.

Precision modes (mm_dtype):
  fp8all — all three gates fp8 e4m3 DoubleRow (2x PE rate: 213ns per
           512-wide 256-K matmul vs 54.6us/gate in bf16). rel err 1.76e-2.
           DEFAULT: ~105us vs 194us for the tuned bf16 baseline.
  mixed  — z/r gates fp8, candidate (tanh) gate bf16. rel err 1.07e-2 but
           ~175us: running fp8 AND bf16 phases on all 8 cores trips a
           package-level clock throttle (~2.07GHz vs 2.37), so the safer
           numerics cost more than their cycle count suggests. Pure-fp8
           does not throttle.
  Scales: weights x256, activations x16; the 1/4096 descale is folded into
  the ACT scale operand. fp8 moving/stationary pairs contract 256 K per
  pass ([128, 2, M] x [128, 2, N]).
"""

import numpy as np

import concourse.bacc as bacc
import concourse.bass as bass
import concourse.mybir as mybir
import concourse.tile as tile
from concourse import bass_utils

P = 128
B = 8192
I = 1024
H = 1024
K = I + H            # 2048 contraction
NCORES = 8
BS = B // NCORES     # 1024 batch rows per core
MT = H // P          # 8 m-tiles (hidden units)
KT = K // P          # 16 k-chunks (bf16)
KT2 = KT // 2        # 8 doubled k-chunks (fp8 DoubleRow)
KH2 = H // P // 2    # 4 doubled chunks in the h/rh half
NFREE = 512          # matmul moving free dim (one PSUM bank of fp32)
NT = BS // NFREE     # 2 n-tiles per core

ASC = 16.0           # activation quantization scale (fp8)
WS = 256.0           # weight quantization scale (fp8)
INV = 1.0 / (ASC * WS)

F32 = mybir.dt.float32
BF16 = mybir.dt.bfloat16
FP8 = mybir.dt.float8e4

AF = mybir.ActivationFunctionType
DR = mybir.MatmulPerfMode.DoubleRow


def build_kernel(mode: str = "fp8all"):
    """Build the per-core Bass kernel. Returns compiled nc."""
    assert mode in ("mixed", "fp8all")
    fp8_h = mode == "fp8all"
    nc = bacc.Bacc("TRN2", target_bir_lowering=False, debug=False)

    # DRAM I/O (per-core shapes).
    x8 = nc.dram_tensor("x8", [I, BS], FP8, kind="ExternalInput").ap()
    h8 = nc.dram_tensor("h8", [H, BS], FP8, kind="ExternalInput").ap()
    hb = nc.dram_tensor("hb", [H, BS], BF16, kind="ExternalInput").ap()
    if not fp8_h:
        xb = nc.dram_tensor("xb", [I, BS], BF16, kind="ExternalInput").ap()
    Wz = nc.dram_tensor("Wz", [MT, P, KT2, 2, P], FP8, kind="ExternalInput").ap()
    Wr = nc.dram_tensor("Wr", [MT, P, KT2, 2, P], FP8, kind="ExternalInput").ap()
    if fp8_h:
        Wh = nc.dram_tensor("Wh", [MT, P, KT2, 2, P], FP8,
                            kind="ExternalInput").ap()
    else:
        Wh = nc.dram_tensor("Wh", [MT, P, K], BF16, kind="ExternalInput").ap()
    bz = nc.dram_tensor("bz", [P, MT], F32, kind="ExternalInput").ap()
    br = nc.dram_tensor("br", [P, MT], F32, kind="ExternalInput").ap()
    bh = nc.dram_tensor("bh", [P, MT], F32, kind="ExternalInput").ap()
    out = nc.dram_tensor("out", [H, BS], BF16, kind="ExternalOutput").ap()

    with tile.TileContext(nc) as tc:
        with (
            tc.tile_pool(name="acts", bufs=1) as acts,
            tc.tile_pool(name="gates", bufs=1) as gates,
            tc.tile_pool(name="wpool", bufs=8) as wpool,
            tc.tile_pool(name="opool", bufs=6) as opool,
            tc.tile_pool(name="ppool", bufs=8, space="PSUM") as ppool,
        ):
            bz_sb = acts.tile([P, MT], F32)
            br_sb = acts.tile([P, MT], F32)
            bh_sb = acts.tile([P, MT], F32)

            # Weight tiles rotate through one 8-buffer pool (fp8all): the
            # 9th+ weight DMA must wait for the matmuls consuming the
            # buffer's previous occupant. This backpressure stops the Wz/Wh
            # prefetch (needed only 40/70us in) from saturating the DMA
            # queues while the r-phase act loads are on the critical path.
            def wtile(nm):
                if fp8_h:
                    return wpool.tile([P, KT2, 2, P], FP8, tag="w", name=nm)
                return wpool.tile([P, KT2, 2, P], FP8, name=nm)

            wr_sb = {mt: wtile(f"wr{mt}") for mt in range(MT)}

            # Persistent activations, feature-major: [p, ko, batch]
            x8_sb = acts.tile([P, I // P, BS], FP8)
            h8_sb = acts.tile([P, H // P, BS], FP8)
            hb_sb = acts.tile([P, H // P, BS], BF16)
            xb_sb = (acts.tile([P, I // P, BS], BF16, name="xb_sb")
                     if not fp8_h else None)
            x8_r = x8.rearrange("(ko p) b -> p ko b", p=P)
            h8_r = h8.rearrange("(ko p) b -> p ko b", p=P)
            hb_r = hb.rearrange("(ko p) b -> p ko b", p=P)
            xb_r = xb.rearrange("(ko p) b -> p ko b", p=P) if not fp8_h else None
            n0 = slice(0, NFREE)
            n1 = slice(NFREE, BS)

            # PE p-state warm-up: the tensor engine runs at reduced clock
            # (~250ns/512-row matmul vs 213) unless warmed; without this
            # block the ENTIRE kernel executes at the lower clock. Chew
            # through the warm-up on garbage matmuls (no DMA deps, results
            # never read) while the real weights/acts are still in flight.
            wu_w = acts.tile([P, 2, P], FP8, name="wu_w")
            wu_m = acts.tile([P, 2, 256], FP8, name="wu_m")
            nc.vector.memset(wu_w[:], 0)
            nc.vector.memset(wu_m[:], 0)
            wu_ps = ppool.tile([P, NFREE], F32, tag="ps", name="wu_ps")
            for i in range(44):
                nc.tensor.matmul(wu_ps[:, :256], wu_w[:], wu_m[:],
                                 start=True, stop=True, perf_mode=DR)

            # Early DMA bandwidth is descriptor-supply-limited PER RING
            # (~175KB/us each, strictly FIFO within a ring), so the
            # ramp-critical bytes are spread over all three rings in
            # consumption order. The scalar ring spins up ~1-3us after
            # sync; gpsimd ~2us after.
            nc.sync.dma_start(wr_sb[0][:], Wr[0])
            nc.sync.dma_start(h8_sb[:, :4, n0], h8_r[:, :4, n0])
            nc.sync.dma_start(h8_sb[:, 4:, n0], h8_r[:, 4:, n0])
            nc.sync.dma_start(x8_sb[:, :, n0], x8_r[:, :, n0])
            nc.sync.dma_start(h8_sb[:, :, n1], h8_r[:, :, n1])
            nc.sync.dma_start(x8_sb[:, :, n1], x8_r[:, :, n1])
            nc.sync.dma_start(hb_sb[:, :4, :], hb_r[:, :4, :])
            nc.sync.dma_start(hb_sb[:, 4:, :], hb_r[:, 4:, :])

            nc.scalar.dma_start(wr_sb[1][:], Wr[1])
            nc.scalar.dma_start(wr_sb[2][:], Wr[2])
            nc.scalar.dma_start(wr_sb[3][:], Wr[3])
            # Pre-warm the ACT sigmoid/tanh table set during the DMA fill so
            # the first real sigmoid doesn't pay the ~2.7us ACT_TABLE_LOAD.
            warm = acts.tile([P, 1], F32)
            nc.scalar.activation(warm[:], warm[:], AF.Sigmoid)
            nc.scalar.dma_start(br_sb[:], br)
            nc.scalar.dma_start(bz_sb[:], bz)
            nc.scalar.dma_start(bh_sb[:], bh)

            # Remaining weights ride the GpSimd SWDGE queue (spins up ~6us,
            # idle otherwise); wz/wh are held back by the wpool rotation.
            for mt in range(4, MT):
                nc.gpsimd.dma_start(wr_sb[mt][:], Wr[mt])
            wz_sb = {}
            for mt in range(MT):
                wz_sb[mt] = wpool.tile([P, KT2, 2, P], FP8,
                                       tag="w" if fp8_h else None,
                                       name=f"wz{mt}")
                nc.gpsimd.dma_start(wz_sb[mt][:], Wz[mt])
            wh_sb = {}
            for mt in range(MT):
                if fp8_h:
                    wh_sb[mt] = wpool.tile([P, KT2, 2, P], FP8, tag="w",
                                           name=f"wh{mt}")
                else:
                    wh_sb[mt] = wpool.tile([P, K], BF16, name=f"wh{mt}")
                nc.gpsimd.dma_start(wh_sb[mt][:], Wh[mt])
            if not fp8_h:
                nc.gpsimd.dma_start(xb_sb[:, :4, :], xb_r[:, :4, :])
                nc.gpsimd.dma_start(xb_sb[:, 4:, :], xb_r[:, 4:, :])

            # Gate results, feature-major. v = (1-z)*h_prev is precomputed
            # during the (PE-bound, DVE-idle) z phase so the final combine
            # after the last matmul is only mul+add.
            z_sb = gates.tile([P, MT, BS], BF16)
            v_sb = gates.tile([P, MT, BS], BF16)
            rh_sb = gates.tile([P, MT, BS], FP8 if fp8_h else BF16)

            def mov8(k2, base, width):
                """fp8 moving operand [128, 2, width] for doubled chunk k2
                of the [h_prev, x] concat."""
                if k2 < KH2:
                    return h8_sb[:, 2 * k2:2 * k2 + 2, base:base + width]
                kx = k2 - KH2
                return x8_sb[:, 2 * kx:2 * kx + 2, base:base + width]

            def mov8_h(k2, base, width):
                """fp8 moving operand for the candidate gate ([rh, x])."""
                if k2 < KH2:
                    return rh_sb[:, 2 * k2:2 * k2 + 2, base:base + width]
                kx = k2 - KH2
                return x8_sb[:, 2 * kx:2 * kx + 2, base:base + width]

            def movb_h(k, base, width):
                """bf16 moving operand [128, width] for the candidate gate."""
                if k < H // P:
                    return rh_sb[:, k, base:base + width]
                return xb_sb[:, k - H // P, base:base + width]

            def fp8_group(ps, w_sb, mov, base, width):
                for k2 in range(KT2):
                    nc.tensor.matmul(
                        ps, w_sb[:, k2], mov(k2, base, width),
                        start=(k2 == 0), stop=(k2 == KT2 - 1), perf_mode=DR)

            # ---- R gate (fp8) ----
            # 8-wide mega-ramp: open ALL 8 PSUM banks on (mt0-7, n=0) and
            # run the entire h_prev half (32 matmuls, ~6.8us of runway)
            # before the first x chunk is touched. This moves the x8-n0
            # deadline from ramp+3.4us to ramp+6.8us — past its DMA arrival
            # even on a slow ring day — and staggers the weight needs to
            # match their ring order: wr0-3 (sync/scalar) first, wr4-7
            # (gpsimd) from ~+3.4us, g6/g7 last.
            NG = 8
            pss = [ppool.tile([P, NFREE], F32, tag="ps", name=f"psri{g}")
                   for g in range(NG)]
            ramp = ([(k2, g) for k2 in range(KH2) for g in range(4)]
                    + [(k2, g) for k2 in range(KH2) for g in (4, 5)]
                    + [(k2, g) for k2 in range(KH2) for g in (6, 7)]
                    + [(k2, g) for k2 in range(KH2, KT2) for g in range(NG)])
            for k2, g in ramp:
                nc.tensor.matmul(
                    pss[g], wr_sb[g][:, k2], mov8(k2, 0, NFREE),
                    start=(k2 == 0), stop=(k2 == KT2 - 1), perf_mode=DR)

            def finish_r(mt, ps, base, width):
                ns = slice(base, base + width)
                r_tmp = opool.tile([P, width], BF16, tag="rt")
                nc.scalar.activation(r_tmp, ps, AF.Sigmoid,
                                     bias=br_sb[:, mt:mt + 1], scale=INV)
                if fp8_h:
                    # rh = fp8(16 * r * h_prev): one fused DVE op
                    nc.vector.scalar_tensor_tensor(
                        rh_sb[:, mt, ns], r_tmp, ASC, hb_sb[:, mt, ns],
                        op0=mybir.AluOpType.mult, op1=mybir.AluOpType.mult)
                else:
                    nc.vector.tensor_mul(
                        rh_sb[:, mt, ns], r_tmp, hb_sb[:, mt, ns])

            for g in range(NG):
                finish_r(g, pss[g], 0, NFREE)
            r_plan = [(mt, 1) for mt in range(MT)]
            for mt, n in r_plan:
                base = n * NFREE
                ps = ppool.tile([P, NFREE], F32, tag="ps", name=f"psr{mt}_{n}")
                fp8_group(ps, wr_sb[mt], mov8, base, NFREE)
                finish_r(mt, ps, base, NFREE)

            # ---- Z gate (fp8) ----
            for mt in range(MT):
                for n in range(NT):
                    base = n * NFREE
                    ns = slice(base, base + NFREE)
                    ps = ppool.tile([P, NFREE], F32, tag="ps",
                                    name=f"psz{mt}_{n}")
                    fp8_group(ps, wz_sb[mt], mov8, base, NFREE)
                    nc.scalar.activation(z_sb[:, mt, ns], ps, AF.Sigmoid,
                                         bias=bz_sb[:, mt:mt + 1], scale=INV)
                    u = opool.tile([P, NFREE], BF16, tag="u")
                    nc.vector.tensor_mul(u, z_sb[:, mt, ns], hb_sb[:, mt, ns])
                    nc.vector.tensor_sub(v_sb[:, mt, ns], hb_sb[:, mt, ns], u)

            # ---- H (candidate) gate ----
            for mt in range(MT):
                for n in range(NT):
                    # Split the very last group so its activation+combine
                    # +store chain pipelines instead of sitting fully
                    # exposed after the final matmul.
                    last = mt == MT - 1 and n == NT - 1
                    nsub = 2 if last else 1
                    width = NFREE // nsub
                    for s in range(nsub):
                        base = n * NFREE + s * width
                        ns = slice(base, base + width)
                        ps = ppool.tile([P, width], F32, tag="ps",
                                        name=f"psh{mt}_{n}_{s}")
                        if fp8_h:
                            fp8_group(ps, wh_sb[mt], mov8_h, base, width)
                        else:
                            for k in range(KT):
                                nc.tensor.matmul(
                                    ps, wh_sb[mt][:, k * P:(k + 1) * P],
                                    movb_h(k, base, width),
                                    start=(k == 0), stop=(k == KT - 1))
                        ht = opool.tile([P, width], BF16, tag="ht",
                                        name=f"ht{mt}_{n}_{s}")
                        nc.scalar.activation(
                            ht, ps, AF.Tanh, bias=bh_sb[:, mt:mt + 1],
                            scale=INV if fp8_h else 1.0)
                        # h = z*h_tilde + v, all bf16
                        nc.vector.tensor_mul(ht, ht, z_sb[:, mt, ns])
                        nc.vector.tensor_add(ht, ht, v_sb[:, mt, ns])
                        # Stores alternate between the sync and scalar HWDGE
                        # rings (both idle by the h phase) so the final
                        # tiles' stores drain in parallel. gpsimd is avoided:
                        # its FIFO still holds backpressured Wh loads.
                        eng = (nc.sync, nc.scalar)[(mt * NT + n + s) % 2]
                        eng.dma_start(out[mt * P:(mt + 1) * P, ns], ht)

    nc.compile()
    return nc


def _prep_inputs(x, h_prev, W_z, b_z, W_r, b_r, W_h, b_h, mode="mixed"):
    """Host-side relayout + quantization."""
    import ml_dtypes
    E4 = ml_dtypes.float8_e4m3
    BF = ml_dtypes.bfloat16
    fp8_h = mode == "fp8all"

    def prep_w8(W):
        # want w[mt, p, k2, i, m] = 256*W[mt*128+m, (k2*2+i)*128+p]
        W5 = W.reshape(MT, P, KT2, 2, P)          # [mt, m, k2, i, p]
        W5 = np.ascontiguousarray(W5.transpose(0, 4, 2, 3, 1))
        return np.clip(W5 * WS, -240, 240).astype(E4)

    def prep_wb(W):
        # want w[mt, p, ko*128+m] = W[mt*128+m, ko*128+p]
        W4 = W.reshape(MT, P, KT, P)              # [mt, m, ko, p]
        return np.ascontiguousarray(
            W4.transpose(0, 3, 2, 1)).reshape(MT, P, K).astype(BF)

    def prep_b(b):
        return np.ascontiguousarray(b.reshape(MT, P).T)

    xT = np.ascontiguousarray(x.T)                # [I, B] f32
    hT = np.ascontiguousarray(h_prev.T)           # [H, B] f32
    x8 = np.clip(xT * ASC, -240, 240).astype(E4)
    h8 = np.clip(hT * ASC, -240, 240).astype(E4)
    xb = xT.astype(BF)
    hb = hT.astype(BF)
    shared = {
        "Wz": prep_w8(W_z), "Wr": prep_w8(W_r),
        "Wh": prep_w8(W_h) if fp8_h else prep_wb(W_h),
        "bz": prep_b(b_z), "br": prep_b(b_r), "bh": prep_b(b_h),
    }
    in_maps = []
    for c in range(NCORES):
        bs = slice(c * BS, (c + 1) * BS)
        m = dict(shared)
        m["x8"] = np.ascontiguousarray(x8[:, bs])
        m["h8"] = np.ascontiguousarray(h8[:, bs])
        m["hb"] = np.ascontiguousarray(hb[:, bs])
        if not fp8_h:
            m["xb"] = np.ascontiguousarray(xb[:, bs])
        in_maps.append(m)
    return in_maps


def run(inputs, mm_dtype="fp8all", trace=False, **run_kwargs):
    """Compile + run on 8 cores. Returns (output [B,H] f32, BassKernelResults)."""
    nc = build_kernel(mm_dtype)
    in_maps = _prep_inputs(**inputs, mode=mm_dtype)
    res = bass_utils.run_bass_kernel_spmd(
        nc, in_maps, core_ids=list(range(NCORES)), trace=trace, **run_kwargs)
    outT = np.concatenate(
        [res.results[c]["out"].astype(np.float32) for c in range(NCORES)],
        axis=1)  # [H, B]
    return np.ascontiguousarray(outT.T), res


def kernel(**inputs) -> np.ndarray:
    import time as _time
    # The axon-tunneled device occasionally reports a transient
    # "unrecoverable" state (NRT_EXEC_UNIT_UNRECOVERABLE) right after a
    # crashed or heavy session; a fresh attempt after a pause recovers.
    # Escalating backoff: 15s, 45s, 90s.
    last = None
    for pause in (0, 15, 45, 90):
        if pause:
            _time.sleep(pause)
        try:
            out, _ = run(inputs)
            return out
        except Exception as e:  # noqa: BLE001
            last = e
    raise last


# revision 46
# speedup vs baseline: 1.0129x; 1.0115x over previous
"""GRU cell kernel for Trainium2, data-parallel across 8 NeuronCores.

Reference computation (per batch row):
    concat = [h_prev, x]                       # [B, 2048]
    z = sigmoid(concat @ W_z.T + b_z)          # [B, 1024]
    r = sigmoid(concat @ W_r.T + b_r)
    h_tilde = tanh([r*h_prev, x] @ W_h.T + b_h)
    h = (1-z)*h_prev + z*h_tilde

Sharding: batch dim (8192) split 1024/core; weights replicated.
Layout on device is feature-major ([feature, batch]) so the batch is the
matmul moving dimension (N=512 per PSUM bank) and the hidden units are the
PSUM partition dim. Host transposes in/out.

Precision modes (mm_dtype):
  fp8all — all three gates fp8 e4m3 DoubleRow (2x PE rate: 213ns per
           512-wide 256-K matmul vs 54.6us/gate in bf16). rel err 1.76e-2.
           DEFAULT: ~105us vs 194us for the tuned bf16 baseline.
  mixed  — z/r gates fp8, candidate (tanh) gate bf16. rel err 1.07e-2 but
           ~175us: running fp8 AND bf16 phases on all 8 cores trips a
           package-level clock throttle (~2.07GHz vs 2.37), so the safer
           numerics cost more than their cycle count suggests. Pure-fp8
           does not throttle.
  Scales: weights x256, activations x16; the 1/4096 descale is folded into
  the ACT scale operand. fp8 moving/stationary pairs contract 256 K per
  pass ([128, 2, M] x [128, 2, N]).
"""

import numpy as np

import concourse.bacc as bacc
import concourse.bass as bass
import concourse.mybir as mybir
import concourse.tile as tile
from concourse import bass_utils

P = 128
B = 8192
I = 1024
H = 1024
K = I + H            # 2048 contraction
NCORES = 8
BS = B // NCORES     # 1024 batch rows per core
MT = H // P          # 8 m-tiles (hidden units)
KT = K // P          # 16 k-chunks (bf16)
KT2 = KT // 2        # 8 doubled k-chunks (fp8 DoubleRow)
KH2 = H // P // 2    # 4 doubled chunks in the h/rh half
NFREE = 512          # matmul moving free dim (one PSUM bank of fp32)
NT = BS // NFREE     # 2 n-tiles per core

ASC = 16.0           # activation quantization scale (fp8)
WS = 256.0           # weight quantization scale (fp8)
INV = 1.0 / (ASC * WS)

F32 = mybir.dt.float32
BF16 = mybir.dt.bfloat16
FP8 = mybir.dt.float8e4

AF = mybir.ActivationFunctionType
DR = mybir.MatmulPerfMode.DoubleRow


def build_kernel(mode: str = "fp8all"):
    """Build the per-core Bass kernel. Returns compiled nc."""
    assert mode in ("mixed", "fp8all")
    fp8_h = mode == "fp8all"
    nc = bacc.Bacc("TRN2", target_bir_lowering=False, debug=False)

    # DRAM I/O (per-core shapes).
    x8 = nc.dram_tensor("x8", [I, BS], FP8, kind="ExternalInput").ap()
    h8 = nc.dram_tensor("h8", [H, BS], FP8, kind="ExternalInput").ap()
    hb = nc.dram_tensor("hb", [H, BS], BF16, kind="ExternalInput").ap()
    if not fp8_h:
        xb = nc.dram_tensor("xb", [I, BS], BF16, kind="ExternalInput").ap()
    Wz = nc.dram_tensor("Wz", [MT, P, KT2, 2, P], FP8, kind="ExternalInput").ap()
    Wr = nc.dram_tensor("Wr", [MT, P, KT2, 2, P], FP8, kind="ExternalInput").ap()
    if fp8_h:
        Wh = nc.dram_tensor("Wh", [MT, P, KT2, 2, P], FP8,
                            kind="ExternalInput").ap()
    else:
        Wh = nc.dram_tensor("Wh", [MT, P, K], BF16, kind="ExternalInput").ap()
    bz = nc.dram_tensor("bz", [P, MT], F32, kind="ExternalInput").ap()
    br = nc.dram_tensor("br", [P, MT], F32, kind="ExternalInput").ap()
    bh = nc.dram_tensor("bh", [P, MT], F32, kind="ExternalInput").ap()
    out = nc.dram_tensor("out", [H, BS], BF16, kind="ExternalOutput").ap()

    with tile.TileContext(nc) as tc:
        with (
            tc.tile_pool(name="acts", bufs=1) as acts,
            tc.tile_pool(name="gates", bufs=1) as gates,
            tc.tile_pool(name="wpool", bufs=8) as wpool,
            tc.tile_pool(name="opool", bufs=6) as opool,
            tc.tile_pool(name="ppool", bufs=8, space="PSUM") as ppool,
        ):
            bz_sb = acts.tile([P, MT], F32)
            br_sb = acts.tile([P, MT], F32)
            bh_sb = acts.tile([P, MT], F32)

            # Weight tiles rotate through one 8-buffer pool (fp8all): the
            # 9th+ weight DMA must wait for the matmuls consuming the
            # buffer's previous occupant. This backpressure stops the Wz/Wh
            # prefetch (needed only 40/70us in) from saturating the DMA
            # queues while the r-phase act loads are on the critical path.
            def wtile(nm):
                if fp8_h:
                    return wpool.tile([P, KT2, 2, P], FP8, tag="w", name=nm)
                return wpool.tile([P, KT2, 2, P], FP8, name=nm)

            wr_sb = {mt: wtile(f"wr{mt}") for mt in range(MT)}

            # Persistent activations, feature-major: [p, ko, batch]
            x8_sb = acts.tile([P, I // P, BS], FP8)
            h8_sb = acts.tile([P, H // P, BS], FP8)
            hb_sb = acts.tile([P, H // P, BS], BF16)
            xb_sb = (acts.tile([P, I // P, BS], BF16, name="xb_sb")
                     if not fp8_h else None)
            x8_r = x8.rearrange("(ko p) b -> p ko b", p=P)
            h8_r = h8.rearrange("(ko p) b -> p ko b", p=P)
            hb_r = hb.rearrange("(ko p) b -> p ko b", p=P)
            xb_r = xb.rearrange("(ko p) b -> p ko b", p=P) if not fp8_h else None
            n0 = slice(0, NFREE)
            n1 = slice(NFREE, BS)

            # PE p-state warm-up: the tensor engine runs at reduced clock
            # (~250ns/512-row matmul vs 213) unless warmed; without this
            # block the ENTIRE kernel executes at the lower clock. Chew
            # through the warm-up on garbage matmuls (no DMA deps, results
            # never read) while the real weights/acts are still in flight.
            wu_w = acts.tile([P, 2, P], FP8, name="wu_w")
            wu_m = acts.tile([P, 2, 256], FP8, name="wu_m")
            nc.vector.memset(wu_w[:], 0)
            nc.vector.memset(wu_m[:], 0)
            wu_ps = ppool.tile([P, NFREE], F32, tag="ps", name="wu_ps")
            for i in range(36):
                nc.tensor.matmul(wu_ps[:, :256], wu_w[:], wu_m[:],
                                 start=True, stop=True, perf_mode=DR)

            # Early DMA bandwidth is descriptor-supply-limited PER RING
            # (~175KB/us each, strictly FIFO within a ring), so the
            # ramp-critical bytes are spread over all three rings in
            # consumption order. The scalar ring spins up ~1-3us after
            # sync; gpsimd ~2us after.
            # Ramp weights load in k2-halves: the mega-ramp's phase A/B
            # (h_prev half) needs only the k2:0-3 halves of wr0-7, so the
            # gating set for the first real matmuls is halved; the k2:4-7
            # halves arrive well before phase C (+6.8us).
            nc.sync.dma_start(wr_sb[0][:, :KH2], Wr[0][:, :KH2])
            nc.sync.dma_start(h8_sb[:, :4, n0], h8_r[:, :4, n0])
            nc.sync.dma_start(wr_sb[0][:, KH2:], Wr[0][:, KH2:])
            nc.sync.dma_start(h8_sb[:, 4:, n0], h8_r[:, 4:, n0])
            nc.sync.dma_start(x8_sb[:, :, n0], x8_r[:, :, n0])
            nc.sync.dma_start(h8_sb[:, :, n1], h8_r[:, :, n1])
            nc.sync.dma_start(x8_sb[:, :, n1], x8_r[:, :, n1])
            nc.sync.dma_start(hb_sb[:, :4, :], hb_r[:, :4, :])
            nc.sync.dma_start(hb_sb[:, 4:, :], hb_r[:, 4:, :])

            for mt in (1, 2, 3):
                nc.scalar.dma_start(wr_sb[mt][:, :KH2], Wr[mt][:, :KH2])
            for mt in (1, 2, 3):
                nc.scalar.dma_start(wr_sb[mt][:, KH2:], Wr[mt][:, KH2:])
            # Pre-warm the ACT sigmoid/tanh table set during the DMA fill so
            # the first real sigmoid doesn't pay the ~2.7us ACT_TABLE_LOAD.
            warm = acts.tile([P, 1], F32)
            nc.scalar.activation(warm[:], warm[:], AF.Sigmoid)
            nc.scalar.dma_start(br_sb[:], br)
            nc.scalar.dma_start(bz_sb[:], bz)
            nc.scalar.dma_start(bh_sb[:], bh)

            # Remaining weights ride the GpSimd SWDGE queue (spins up ~6us,
            # idle otherwise); wz/wh are held back by the wpool rotation.
            for mt in range(4, MT):
                nc.gpsimd.dma_start(wr_sb[mt][:, :KH2], Wr[mt][:, :KH2])
            for mt in range(4, MT):
                nc.gpsimd.dma_start(wr_sb[mt][:, KH2:], Wr[mt][:, KH2:])
            wz_sb = {}
            for mt in range(MT):
                wz_sb[mt] = wpool.tile([P, KT2, 2, P], FP8,
                                       tag="w" if fp8_h else None,
                                       name=f"wz{mt}")
                nc.gpsimd.dma_start(wz_sb[mt][:], Wz[mt])
            wh_sb = {}
            for mt in range(MT):
                if fp8_h:
                    wh_sb[mt] = wpool.tile([P, KT2, 2, P], FP8, tag="w",
                                           name=f"wh{mt}")
                else:
                    wh_sb[mt] = wpool.tile([P, K], BF16, name=f"wh{mt}")
                nc.gpsimd.dma_start(wh_sb[mt][:], Wh[mt])
            if not fp8_h:
                nc.gpsimd.dma_start(xb_sb[:, :4, :], xb_r[:, :4, :])
                nc.gpsimd.dma_start(xb_sb[:, 4:, :], xb_r[:, 4:, :])

            # Gate results, feature-major. v = (1-z)*h_prev is precomputed
            # during the (PE-bound, DVE-idle) z phase so the final combine
            # after the last matmul is only mul+add.
            z_sb = gates.tile([P, MT, BS], BF16)
            v_sb = gates.tile([P, MT, BS], BF16)
            rh_sb = gates.tile([P, MT, BS], FP8 if fp8_h else BF16)

            def mov8(k2, base, width):
                """fp8 moving operand [128, 2, width] for doubled chunk k2
                of the [h_prev, x] concat."""
                if k2 < KH2:
                    return h8_sb[:, 2 * k2:2 * k2 + 2, base:base + width]
                kx = k2 - KH2
                return x8_sb[:, 2 * kx:2 * kx + 2, base:base + width]

            def mov8_h(k2, base, width):
                """fp8 moving operand for the candidate gate ([rh, x])."""
                if k2 < KH2:
                    return rh_sb[:, 2 * k2:2 * k2 + 2, base:base + width]
                kx = k2 - KH2
                return x8_sb[:, 2 * kx:2 * kx + 2, base:base + width]

            def movb_h(k, base, width):
                """bf16 moving operand [128, width] for the candidate gate."""
                if k < H // P:
                    return rh_sb[:, k, base:base + width]
                return xb_sb[:, k - H // P, base:base + width]

            def fp8_group(ps, w_sb, mov, base, width):
                for k2 in range(KT2):
                    nc.tensor.matmul(
                        ps, w_sb[:, k2], mov(k2, base, width),
                        start=(k2 == 0), stop=(k2 == KT2 - 1), perf_mode=DR)

            # ---- R gate (fp8) ----
            # 8-wide mega-ramp: open ALL 8 PSUM banks on (mt0-7, n=0) and
            # run the entire h_prev half (32 matmuls, ~6.8us of runway)
            # before the first x chunk is touched. This moves the x8-n0
            # deadline from ramp+3.4us to ramp+6.8us — past its DMA arrival
            # even on a slow ring day — and staggers the weight needs to
            # match their ring order: wr0-3 (sync/scalar) first, wr4-7
            # (gpsimd) from ~+3.4us, g6/g7 last.
            NG = 8
            pss = [ppool.tile([P, NFREE], F32, tag="ps", name=f"psri{g}")
                   for g in range(NG)]
            ramp = ([(k2, g) for k2 in range(KH2) for g in range(4)]
                    + [(k2, g) for k2 in range(KH2) for g in (4, 5)]
                    + [(k2, g) for k2 in range(KH2) for g in (6, 7)]
                    + [(k2, g) for k2 in range(KH2, KT2) for g in range(NG)])
            for k2, g in ramp:
                nc.tensor.matmul(
                    pss[g], wr_sb[g][:, k2], mov8(k2, 0, NFREE),
                    start=(k2 == 0), stop=(k2 == KT2 - 1), perf_mode=DR)

            def finish_r(mt, ps, base, width):
                ns = slice(base, base + width)
                r_tmp = opool.tile([P, width], BF16, tag="rt")
                nc.scalar.activation(r_tmp, ps, AF.Sigmoid,
                                     bias=br_sb[:, mt:mt + 1], scale=INV)
                if fp8_h:
                    # rh = fp8(16 * r * h_prev): one fused DVE op
                    nc.vector.scalar_tensor_tensor(
                        rh_sb[:, mt, ns], r_tmp, ASC, hb_sb[:, mt, ns],
                        op0=mybir.AluOpType.mult, op1=mybir.AluOpType.mult)
                else:
                    nc.vector.tensor_mul(
                        rh_sb[:, mt, ns], r_tmp, hb_sb[:, mt, ns])

            for g in range(NG):
                finish_r(g, pss[g], 0, NFREE)
            r_plan = [(mt, 1) for mt in range(MT)]
            for mt, n in r_plan:
                base = n * NFREE
                ps = ppool.tile([P, NFREE], F32, tag="ps", name=f"psr{mt}_{n}")
                fp8_group(ps, wr_sb[mt], mov8, base, NFREE)
                finish_r(mt, ps, base, NFREE)

            # ---- Z gate (fp8) ----
            for mt in range(MT):
                for n in range(NT):
                    base = n * NFREE
                    ns = slice(base, base + NFREE)
                    ps = ppool.tile([P, NFREE], F32, tag="ps",
                                    name=f"psz{mt}_{n}")
                    fp8_group(ps, wz_sb[mt], mov8, base, NFREE)
                    nc.scalar.activation(z_sb[:, mt, ns], ps, AF.Sigmoid,
                                         bias=bz_sb[:, mt:mt + 1], scale=INV)
                    u = opool.tile([P, NFREE], BF16, tag="u")
                    nc.vector.tensor_mul(u, z_sb[:, mt, ns], hb_sb[:, mt, ns])
                    nc.vector.tensor_sub(v_sb[:, mt, ns], hb_sb[:, mt, ns], u)

            # ---- H (candidate) gate ----
            for mt in range(MT):
                for n in range(NT):
                    # Split the very last group so its activation+combine
                    # +store chain pipelines instead of sitting fully
                    # exposed after the final matmul.
                    last = mt == MT - 1 and n == NT - 1
                    nsub = 2 if last else 1
                    width = NFREE // nsub
                    for s in range(nsub):
                        base = n * NFREE + s * width
                        ns = slice(base, base + width)
                        ps = ppool.tile([P, width], F32, tag="ps",
                                        name=f"psh{mt}_{n}_{s}")
                        if fp8_h:
                            fp8_group(ps, wh_sb[mt], mov8_h, base, width)
                        else:
                            for k in range(KT):
                                nc.tensor.matmul(
                                    ps, wh_sb[mt][:, k * P:(k + 1) * P],
                                    movb_h(k, base, width),
                                    start=(k == 0), stop=(k == KT - 1))
                        ht = opool.tile([P, width], BF16, tag="ht",
                                        name=f"ht{mt}_{n}_{s}")
                        nc.scalar.activation(
                            ht, ps, AF.Tanh, bias=bh_sb[:, mt:mt + 1],
                            scale=INV if fp8_h else 1.0)
                        # h = z*h_tilde + v, all bf16
                        nc.vector.tensor_mul(ht, ht, z_sb[:, mt, ns])
                        nc.vector.tensor_add(ht, ht, v_sb[:, mt, ns])
                        # Stores alternate between the sync and scalar HWDGE
                        # rings (both idle by the h phase) so the final
                        # tiles' stores drain in parallel. gpsimd is avoided:
                        # its FIFO still holds backpressured Wh loads.
                        eng = (nc.sync, nc.scalar)[(mt * NT + n + s) % 2]
                        eng.dma_start(out[mt * P:(mt + 1) * P, ns], ht)

    nc.compile()
    return nc


def _prep_inputs(x, h_prev, W_z, b_z, W_r, b_r, W_h, b_h, mode="mixed"):
    """Host-side relayout + quantization."""
    import ml_dtypes
    E4 = ml_dtypes.float8_e4m3
    BF = ml_dtypes.bfloat16
    fp8_h = mode == "fp8all"

    def prep_w8(W):
        # want w[mt, p, k2, i, m] = 256*W[mt*128+m, (k2*2+i)*128+p]
        W5 = W.reshape(MT, P, KT2, 2, P)          # [mt, m, k2, i, p]
        W5 = np.ascontiguousarray(W5.transpose(0, 4, 2, 3, 1))
        return np.clip(W5 * WS, -240, 240).astype(E4)

    def prep_wb(W):
        # want w[mt, p, ko*128+m] = W[mt*128+m, ko*128+p]
        W4 = W.reshape(MT, P, KT, P)              # [mt, m, ko, p]
        return np.ascontiguousarray(
            W4.transpose(0, 3, 2, 1)).reshape(MT, P, K).astype(BF)

    def prep_b(b):
        return np.ascontiguousarray(b.reshape(MT, P).T)

    xT = np.ascontiguousarray(x.T)                # [I, B] f32
    hT = np.ascontiguousarray(h_prev.T)           # [H, B] f32
    x8 = np.clip(xT * ASC, -240, 240).astype(E4)
    h8 = np.clip(hT * ASC, -240, 240).astype(E4)
    xb = xT.astype(BF)
    hb = hT.astype(BF)
    shared = {
        "Wz": prep_w8(W_z), "Wr": prep_w8(W_r),
        "Wh": prep_w8(W_h) if fp8_h else prep_wb(W_h),
        "bz": prep_b(b_z), "br": prep_b(b_r), "bh": prep_b(b_h),
    }
    in_maps = []
    for c in range(NCORES):
        bs = slice(c * BS, (c + 1) * BS)
        m = dict(shared)
        m["x8"] = np.ascontiguousarray(x8[:, bs])
        m["h8"] = np.ascontiguousarray(h8[:, bs])
        m["hb"] = np.ascontiguousarray(hb[:, bs])
        if not fp8_h:
            m["xb"] = np.ascontiguousarray(xb[:, bs])
        in_maps.append(m)
    return in_maps


def run(inputs, mm_dtype="fp8all", trace=False, **run_kwargs):
    """Compile + run on 8 cores. Returns (output [B,H] f32, BassKernelResults)."""
    nc = build_kernel(mm_dtype)
    in_maps = _prep_inputs(**inputs, mode=mm_dtype)
    res = bass_utils.run_bass_kernel_spmd(
        nc, in_maps, core_ids=list(range(NCORES)), trace=trace, **run_kwargs)
    outT = np.concatenate(
        [res.results[c]["out"].astype(np.float32) for c in range(NCORES)],
        axis=1)  # [H, B]
    return np.ascontiguousarray(outT.T), res


def kernel(**inputs) -> np.ndarray:
    import time as _time
    # The axon-tunneled device occasionally reports a transient
    # "unrecoverable" state (NRT_EXEC_UNIT_UNRECOVERABLE) right after a
    # crashed or heavy session; a fresh attempt after a pause recovers.
    # Escalating backoff: 15s, 45s, 90s.
    last = None
    for pause in (0, 15, 45, 90):
        if pause:
            _time.sleep(pause)
        try:
            out, _ = run(inputs)
            return out
        except Exception as e:  # noqa: BLE001
            last = e
    raise last
